# revision 19
# baseline (speedup 1.0000x reference)
"""Causal multi-head attention (RoPE) Trainium2 Bass kernel.

Problem: x[2,2048,1024] @ Wq/Wk/Wv -> 16 heads of causal attention with
interleaved-pair RoPE -> @ Wo.  Sharded over 8 NeuronCores as
(batch x head-group): core c handles batch c//4 and heads [4*(c%4), 4*(c%4)+4).
Each core computes a partial y^T = (attn_out_heads @ Wo[rows]) for its head
group; the host sums the 4 partials per batch and transposes back.

On-device layout is "transposed" throughout: x^T, Q^T, K^T live as
[dims, seq] so every matmul contracts over the partition axis.  Scores are
computed transposed (S^T[kv, q]); the softmax denominator comes from an extra
ones-column appended to V; softmax runs without max-subtraction (scaled
scores are O(5), exp cannot overflow in fp32).

Performance structure (vs the fp32r baseline):
  * all matmul operands are bf16 (fp32 PSUM accumulate) - removes the fp32r
    narrow-tile penalty, halves LDWEIGHTS, halves input DMA;
  * dedicated PSUM pools (scores 2x2 banks / PV 2x1 / proj+out 2x1) so the
    static Tile schedule can overlap next-chunk projections with attention
    and with the softmax-normalize tail;
  * normalize chain has no DMA round-trip: DVE copy of the denominator row,
    gpsimd partition_broadcast, DVE reciprocal, DVE multiply;
  * causal diag masking is an extra accumulating 128x128 matmul
    (psum += maskT.T @ I adds the -1e30 upper triangle) instead of DVE adds;
  * score matmuls for the two heads of a partition block are issued
    back-to-back with row tile_position 0/64 so they run concurrently in
    the PE array;
  * out-proj of chunk qc is issued after the projections of chunk qc+1,
    hiding the normalize latency under dense PE work.
"""

import numpy as np
import ml_dtypes

import concourse.bass as bass
import concourse.mybir as mybir
from concourse import bacc
from concourse.tile import TileContext
from concourse.bass_utils import run_bass_kernel_spmd

P = 128          # partitions
S = 2048         # sequence length
D = 1024         # model dim
DK = 64          # head dim
HPC = 4          # heads per core
DH = HPC * DK    # qkv dims per core (256)
KO = D // P      # 8 contraction slices
QCW = 512        # q chunk width
NQC = S // QCW   # 4 q chunks
NST = S // P     # 16 kv tiles
NCORES = 8

F32 = mybir.dt.float32
BF16 = mybir.dt.bfloat16
EXP = mybir.ActivationFunctionType.Exp
MUL = mybir.AluOpType.mult
ADD = mybir.AluOpType.add
BF = ml_dtypes.bfloat16


def build_nc():
    nc = bacc.Bacc()

    xt = nc.dram_tensor("xt", [D, S], BF16, kind="ExternalInput")
    wq = nc.dram_tensor("wq", [D, DH], BF16, kind="ExternalInput")
    wk = nc.dram_tensor("wk", [D, DH], BF16, kind="ExternalInput")
    wv = nc.dram_tensor("wv", [D, DH], BF16, kind="ExternalInput")
    wo = nc.dram_tensor("wo", [DH, D], BF16, kind="ExternalInput")
    cosd = nc.dram_tensor("cosd", [P, S], BF16, kind="ExternalInput")
    sind = nc.dram_tensor("sind", [P, S], BF16, kind="ExternalInput")
    pswap = nc.dram_tensor("pswap", [P, P], BF16, kind="ExternalInput")
    # maskTd[c, p] = -1e30 if p > c else 0  (transposed lower-exclusive tri)
    maskTd = nc.dram_tensor("maskTd", [P, P], BF16, kind="ExternalInput")
    identd = nc.dram_tensor("identd", [P, P], BF16, kind="ExternalInput")
    onesd = nc.dram_tensor("onesd", [P, NST * HPC], BF16, kind="ExternalInput")
    yt = nc.dram_tensor("yt", [D, S], BF16, kind="ExternalOutput")

    with TileContext(nc) as tc:
        with (
            tc.tile_pool(name="const", bufs=1) as cp,
            tc.tile_pool(name="qt", bufs=2) as qtp,
            tc.tile_pool(name="ot", bufs=2) as otp,
            tc.tile_pool(name="exs", bufs=4) as exp_pool,
            tc.tile_pool(name="raw", bufs=3) as rawp,
            tc.tile_pool(name="rope", bufs=4) as ropep,
            tc.tile_pool(name="den", bufs=4) as denp,
            tc.tile_pool(name="rb", bufs=4) as rbp,
            tc.tile_pool(name="ysb", bufs=3) as ysbp,
            tc.tile_pool(name="psS", bufs=1, space="PSUM") as psS,
            tc.tile_pool(name="psV", bufs=2, space="PSUM") as psV,
            tc.tile_pool(name="psP", bufs=2, space="PSUM") as psP,
        ):
            # ---- constant loads ----
            # sync queue: wk then the x slices (these gate the first matmuls);
            # gpsimd queue: everything else, in parallel.
            # x^T loaded column-chunk-major, one descriptor per chunk, so
            # chunk 0's projections start after ~1/4 of the x DMA; weights go
            # on the gpsimd queue so they transfer in parallel with x.
            xt_sb = cp.tile([P, KO, S], BF16, tag="xt")
            for qc in range(NQC):
                qs = slice(QCW * qc, QCW * (qc + 1))
                nc.sync.dma_start(
                    xt_sb[:, :, qs],
                    xt[:, qs].rearrange("(ko p) s -> p ko s", p=P),
                )
            wk_sb = cp.tile([P, KO, DH], BF16, tag="wk")
            nc.gpsimd.dma_start(wk_sb[:], wk[:].rearrange("(ko p) m -> p ko m", p=P))
            wv_sb = cp.tile([P, KO, DH], BF16, tag="wv")
            nc.gpsimd.dma_start(wv_sb[:], wv[:].rearrange("(ko p) m -> p ko m", p=P))
            wq_sb = cp.tile([P, KO, DH], BF16, tag="wq")
            nc.gpsimd.dma_start(wq_sb[:], wq[:].rearrange("(ko p) m -> p ko m", p=P))
            cos_sb = cp.tile([P, S], BF16, tag="cos")
            nc.gpsimd.dma_start(cos_sb[:], cosd[:])
            sin_sb = cp.tile([P, S], F32, tag="sin")
            nc.gpsimd.dma_start(sin_sb[:], sind[:])
            sw_sb = cp.tile([P, P], BF16, tag="pswap")
            nc.gpsimd.dma_start(sw_sb[:], pswap[:])
            maskT_sb = cp.tile([P, P], BF16, tag="maskT")
            nc.gpsimd.dma_start(maskT_sb[:], maskTd[:])
            ident_sb = cp.tile([P, P], BF16, tag="ident")
            nc.gpsimd.dma_start(ident_sb[:], identd[:])
            wo_sb = cp.tile([P, 2, D], BF16, tag="wo")
            nc.gpsimd.dma_start(wo_sb[:], wo[:].rearrange("(ko p) n -> p ko n", p=P))

            # V in [s-rows, dims] layout with a ones column per head:
            # [V(64) | 1] -> PV out rows 0..63 = O^T, row 64 = sum(exp)
            v_sb = cp.tile([P, NST, HPC, DK + 1], BF16, tag="v")
            nc.gpsimd.dma_start(
                v_sb[:, :, :, DK], onesd[:].rearrange("p (s h) -> p s h", s=NST)
            )

            kt_sb = cp.tile([P, 2, S], BF16, tag="kt")

            def proj_rope(w_sb, dst, pb, qc):
                """dst <- rope( (x @ W)^T )[128 dims block pb, 512 q cols qc]."""
                qs = slice(QCW * qc, QCW * (qc + 1))
                ps = psP.tile([P, QCW], F32, tag="pp")
                for ko in range(KO):
                    nc.tensor.matmul(
                        ps[:],
                        w_sb[:, ko, P * pb : P * (pb + 1)],
                        xt_sb[:, ko, qs],
                        start=(ko == 0),
                        stop=(ko == KO - 1),
                    )
                raw = rawp.tile([P, QCW], BF16, tag="raw")
                nc.vector.tensor_copy(raw[:], ps[:])
                swp = psP.tile([P, QCW], F32, tag="pp")
                nc.tensor.matmul(swp[:], sw_sb[:], raw[:], start=True, stop=True)
                d1 = ropep.tile([P, QCW], BF16, tag="d1")
                nc.vector.tensor_tensor(d1[:], raw[:], cos_sb[:, qs], MUL)
                d2 = ropep.tile([P, QCW], BF16, tag="d2")
                nc.vector.tensor_tensor(d2[:], swp[:], sin_sb[:, qs], MUL)
                nc.vector.tensor_tensor(dst, d1[:], d2[:], ADD)

            def v_proj(st):
                ps = psP.tile([P, QCW], F32, tag="pp")
                for ko in range(KO):
                    nc.tensor.matmul(
                        ps[:, :DH],
                        xt_sb[:, ko, P * st : P * (st + 1)],
                        wv_sb[:, ko, :],
                        start=(ko == 0),
                        stop=(ko == KO - 1),
                    )
                vv = ps[:, :DH].rearrange("p (h d) -> p h d", h=HPC)
                nc.vector.tensor_copy(v_sb[:, st, :, 0:DK], vv)

            def attn(qc):
                """Attention for q chunk qc; returns the bf16 O^T tile."""
                nst = 4 * qc + 4
                ot_t = otp.tile([P, 2, QCW], BF16, tag="ot")
                for hp in range(2):          # partition block = head pair
                    pvs = []
                    for i in range(2):
                        pvs.append(
                            psV.tile([DK + 1, QCW], F32, tag="pv", name=f"pv{i}")
                        )
                    for g in range(nst // 2):
                        sc = psS.tile([P, 2, 2, QCW], F32, tag="sc")
                        rr = []
                        for j in range(2):
                            st = 2 * g + j
                            r = max(P * st - QCW * qc, 0)
                            rr.append(r)
                            for i in range(2):   # head 2hp+i, rows 64i:64i+64
                                nc.tensor.matmul(
                                    sc[:, i, j, r:QCW],
                                    kt_sb[
                                        DK * i : DK * (i + 1),
                                        hp,
                                        P * st : P * (st + 1),
                                    ],
                                    qt_t[DK * i : DK * (i + 1), hp, r:QCW],
                                    start=True,
                                    stop=True,
                                )
                        for j in range(2):
                            st = 2 * g + j
                            r = P * st - QCW * qc
                            if r >= 0:   # diagonal tile: additive triangle mask
                                for i in range(2):
                                    nc.tensor.matmul(
                                        sc[:, i, j, r : r + P],
                                        maskT_sb[:],
                                        ident_sb[:],
                                        start=False,
                                        stop=True,
                                        skip_group_check=True,
                                    )
                        # one exp over all 4 banks; masked-out columns hold
                        # garbage but are never read by the PV matmuls
                        ex = exp_pool.tile([P, 2, 2, QCW], BF16, tag="ex")
                        nc.scalar.activation(
                            ex[:].rearrange("p a b c -> p (a b c)"),
                            sc[:].rearrange("p a b c -> p (a b c)"),
                            EXP,
                            scale=0.125,
                        )
                        for j in range(2):
                            st = 2 * g + j
                            r = rr[j]
                            for i in range(2):
                                nc.tensor.matmul(
                                    pvs[i][:, r:QCW],
                                    v_sb[:, st, 2 * hp + i, :],
                                    ex[:, i, j, r:QCW],
                                    start=(st == 0),
                                    stop=(st == nst - 1),
                                )
                    # normalize: O / sum(exp).  DVE reciprocal is ~8 cyc/elem,
                    # so spread the 512 sums across partitions before the
                    # reciprocal.  Steady state uses an SBUF-to-SBUF DMA
                    # round-trip (cheap on DVE, latency hidden by the next
                    # chunk's projections); the last chunk has nothing left to
                    # hide under, so it uses the lower-latency DVE 32x32
                    # block-transpose spread instead.
                    for i in range(2):
                        if qc < NQC - 1:
                            den = denp.tile([1, QCW], F32, tag="den")
                            nc.vector.tensor_copy(den[:], pvs[i][DK : DK + 1, :])
                            s128 = denp.tile([P, 4], F32, tag="s128")
                            nc.gpsimd.dma_start(s128[:], den[:])
                            r128 = denp.tile([P, 4], F32, tag="r128")
                            nc.vector.reciprocal(r128[:], s128[:])
                            rc = denp.tile([1, QCW], F32, tag="rc")
                            nc.gpsimd.dma_start(rc[:], r128[:])
                        else:
                            den = denp.tile([32, QCW], F32, tag="dent")
                            nc.vector.tensor_copy(
                                den[0:1, :], pvs[i][DK : DK + 1, :]
                            )
                            dT = denp.tile([32, QCW], F32, tag="dT")
                            nc.vector.transpose(dT[:], den[:])
                            rT = denp.tile([32, QCW], F32, tag="rT")
                            nc.vector.reciprocal(
                                rT[:].rearrange("p (a b) -> p a b", b=32)[:, :, 0],
                                dT[:].rearrange("p (a b) -> p a b", b=32)[:, :, 0],
                            )
                            rc = denp.tile([32, QCW], F32, tag="rct")
                            nc.vector.transpose(rc[:], rT[:])
                            rc = rc[0:1, :]
                        rb = rbp.tile([DK, QCW], F32, tag="rb")
                        nc.gpsimd.partition_broadcast(rb[:], rc[:])
                        nc.vector.tensor_tensor(
                            ot_t[DK * i : DK * (i + 1), hp, :],
                            pvs[i][0:DK, :],
                            rb[:],
                            MUL,
                        )
                return ot_t

            def out_proj(qc, ot_t):
                qs = slice(QCW * qc, QCW * (qc + 1))
                for mt in range(KO):
                    yp = psP.tile([P, QCW], F32, tag="pp")
                    for kb in range(2):
                        nc.tensor.matmul(
                            yp[:],
                            wo_sb[:, kb, P * mt : P * (mt + 1)],
                            ot_t[:, kb, :],
                            start=(kb == 0),
                            stop=(kb == 1),
                        )
                    ys = ysbp.tile([P, QCW], BF16, tag="ys")
                    nc.any.tensor_copy(out=ys[:], in_=yp[:])
                    nc.sync.dma_start(yt[P * mt : P * (mt + 1), qs], ys[:])

            # ---- per-q-chunk schedule ----
            ots = {}
            for qc in range(NQC):
                qs = slice(QCW * qc, QCW * (qc + 1))
                for pb in range(2):
                    proj_rope(wk_sb, kt_sb[:, pb, qs], pb, qc)
                for st in range(4 * qc, 4 * qc + 4):
                    v_proj(st)
                qt_t = qtp.tile([P, 2, QCW], BF16, tag="qt")
                for pb in range(2):
                    proj_rope(wq_sb, qt_t[:, pb, :], pb, qc)
                if qc > 0:
                    out_proj(qc - 1, ots.pop(qc - 1))
                ots[qc] = attn(qc)
            out_proj(NQC - 1, ots.pop(NQC - 1))

    nc.finalize()
    return nc


_NC_CACHE = []
_LAST_IN_MAPS = []


def _rope_tables(token_positions):
    pos = np.asarray(token_positions).astype(np.float32)
    exponent = np.arange(0, DK, 2, dtype=np.float32)
    inv_freq = (1.0 / (10000.0 ** (exponent / DK))).astype(np.float32)
    freqs = pos[:, None] * inv_freq[None, :]          # [S, 32]
    cos64 = np.repeat(np.cos(freqs).T.astype(np.float32), 2, axis=0)  # [64, S]
    sin64 = np.repeat(np.sin(freqs).T.astype(np.float32), 2, axis=0)
    sgn = np.where(np.arange(DK) % 2 == 0, -1.0, 1.0).astype(np.float32)
    sin64 = sin64 * sgn[:, None]
    cos128 = np.tile(cos64, (2, 1)).astype(np.float32)
    sin128 = np.tile(sin64, (2, 1)).astype(np.float32)
    return np.ascontiguousarray(cos128), np.ascontiguousarray(sin128)


def prep_in_maps(x, Wq, Wk, Wv, Wo, token_positions):
    x = np.asarray(x, dtype=np.float32)
    b = x.shape[0]

    cos128, sin128 = _rope_tables(token_positions)
    cos128 = cos128.astype(BF)
    sin128 = sin128.astype(BF)

    psw = np.zeros((P, P), dtype=BF)
    idx = np.arange(P)
    psw[idx, idx ^ 1] = 1.0  # swap adjacent pairs

    # maskT[c, p] = -1e30 where p > c (adds upper-kv triangle mask via
    # psum += maskT.T @ I)
    maskT = np.where(
        np.arange(P)[None, :] > np.arange(P)[:, None], -1e30, 0.0
    ).astype(BF)
    ident = np.eye(P, dtype=BF)
    onesd = np.ones((P, NST * HPC), dtype=BF)

    xts = [np.ascontiguousarray(x[bi].T).astype(BF) for bi in range(b)]
    Wq = np.asarray(Wq, dtype=np.float32).astype(BF)
    Wk = np.asarray(Wk, dtype=np.float32).astype(BF)
    Wv = np.asarray(Wv, dtype=np.float32).astype(BF)
    Wo = np.asarray(Wo, dtype=np.float32).astype(BF)

    in_maps = []
    cpb = NCORES // b  # cores per batch
    for c in range(NCORES):
        bi, g = c // cpb, c % 4
        cs = slice(DH * g, DH * (g + 1))
        in_maps.append(
            {
                "xt": xts[bi],
                "wq": np.ascontiguousarray(Wq[:, cs]),
                "wk": np.ascontiguousarray(Wk[:, cs]),
                "wv": np.ascontiguousarray(Wv[:, cs]),
                "wo": np.ascontiguousarray(Wo[cs, :]),
                "cosd": cos128,
                "sind": sin128,
                "pswap": psw,
                "maskTd": maskT,
                "identd": ident,
                "onesd": onesd,
            }
        )
    return in_maps


def kernel(x, Wq, Wk, Wv, Wo, token_positions):
    b = np.asarray(x).shape[0]
    in_maps = prep_in_maps(x, Wq, Wk, Wv, Wo, token_positions)

    if not _NC_CACHE:
        _NC_CACHE.append(build_nc())
    nc = _NC_CACHE[0]
    _LAST_IN_MAPS.clear()
    _LAST_IN_MAPS.append(in_maps)

    res = run_bass_kernel_spmd(nc, in_maps, list(range(NCORES)), trace=False)

    y = np.zeros((b, S, D), dtype=np.float32)
    cpb = NCORES // b
    for c in range(NCORES):
        y[c // cpb] += res.results[c]["yt"].T.astype(np.float32)
    return y


# revision 24
# speedup vs baseline: 1.1909x; 1.1909x over previous
"""Causal multi-head attention (RoPE) Trainium2 Bass kernel.

Problem: x[2,2048,1024] @ Wq/Wk/Wv -> 16 heads of causal attention with
interleaved-pair RoPE -> @ Wo.  Sharded over 8 NeuronCores as
(batch x head-group): core c handles batch c//4 and heads [4*(c%4), 4*(c%4)+4).
Each core computes a partial y^T = (attn_out_heads @ Wo[rows]) for its head
group; the host sums the 4 partials per batch and transposes back.

On-device layout is "transposed" throughout: x^T, Q^T, K^T live as
[dims, seq] so every matmul contracts over the partition axis.  Scores are
computed transposed (S^T[kv, q]); the softmax denominator comes from an extra
ones-column appended to V; softmax runs without max-subtraction (scaled
scores are O(5), exp cannot overflow in fp32).

Performance structure (vs the fp32r baseline):
  * all matmul operands are bf16 (fp32 PSUM accumulate) - removes the fp32r
    narrow-tile penalty, halves LDWEIGHTS, halves input DMA;
  * dedicated PSUM pools (scores 2x2 banks / PV 2x1 / proj+out 2x1) so the
    static Tile schedule can overlap next-chunk projections with attention
    and with the softmax-normalize tail;
  * normalize chain has no DMA round-trip: DVE copy of the denominator row,
    gpsimd partition_broadcast, DVE reciprocal, DVE multiply;
  * causal diag masking is an extra accumulating 128x128 matmul
    (psum += maskT.T @ I adds the -1e30 upper triangle) instead of DVE adds;
  * score matmuls for the two heads of a partition block are issued
    back-to-back with row tile_position 0/64 so they run concurrently in
    the PE array;
  * out-proj of chunk qc is issued after the projections of chunk qc+1,
    hiding the normalize latency under dense PE work.
"""

import numpy as np
import ml_dtypes

import concourse.bass as bass
import concourse.mybir as mybir
from concourse import bacc
from concourse.tile import TileContext
from concourse.bass_utils import run_bass_kernel_spmd

P = 128          # partitions
S = 2048         # sequence length
D = 1024         # model dim
DK = 64          # head dim
HPC = 4          # heads per core
DH = HPC * DK    # qkv dims per core (256)
KO = D // P      # 8 contraction slices
QCW = 512        # q chunk width
NQC = S // QCW   # 4 q chunks
NST = S // P     # 16 kv tiles
NCORES = 8

F32 = mybir.dt.float32
BF16 = mybir.dt.bfloat16
EXP = mybir.ActivationFunctionType.Exp
MUL = mybir.AluOpType.mult
ADD = mybir.AluOpType.add
BF = ml_dtypes.bfloat16


def build_nc():
    nc = bacc.Bacc()

    xt = nc.dram_tensor("xt", [D, S], BF16, kind="ExternalInput")
    wq = nc.dram_tensor("wq", [D, DH], BF16, kind="ExternalInput")
    wk = nc.dram_tensor("wk", [D, DH], BF16, kind="ExternalInput")
    wv = nc.dram_tensor("wv", [D, DH], BF16, kind="ExternalInput")
    wo = nc.dram_tensor("wo", [DH, D], BF16, kind="ExternalInput")
    cosd = nc.dram_tensor("cosd", [P, S], BF16, kind="ExternalInput")
    sind = nc.dram_tensor("sind", [P, S], BF16, kind="ExternalInput")
    pswap = nc.dram_tensor("pswap", [P, P], BF16, kind="ExternalInput")
    # maskTd[c, p] = -1e30 if p > c else 0  (transposed lower-exclusive tri)
    maskTd = nc.dram_tensor("maskTd", [P, P], BF16, kind="ExternalInput")
    identd = nc.dram_tensor("identd", [P, P], BF16, kind="ExternalInput")
    onesd = nc.dram_tensor("onesd", [P, NST * HPC], BF16, kind="ExternalInput")
    yt = nc.dram_tensor("yt", [D, S], BF16, kind="ExternalOutput")

    with TileContext(nc) as tc:
        with (
            tc.tile_pool(name="const", bufs=1) as cp,
            tc.tile_pool(name="qt", bufs=2) as qtp,
            tc.tile_pool(name="ot", bufs=2) as otp,
            tc.tile_pool(name="exs", bufs=4) as exp_pool,
            tc.tile_pool(name="raw", bufs=3) as rawp,
            tc.tile_pool(name="rope", bufs=4) as ropep,
            tc.tile_pool(name="den", bufs=4) as denp,
            tc.tile_pool(name="rb", bufs=4) as rbp,
            tc.tile_pool(name="ysb", bufs=3) as ysbp,
            tc.tile_pool(name="psS", bufs=2, space="PSUM") as psS,
            tc.tile_pool(name="psV", bufs=2, space="PSUM") as psV,
            tc.tile_pool(name="psP", bufs=2, space="PSUM") as psP,
        ):
            # ---- constant loads ----
            # sync queue: wk then the x slices (these gate the first matmuls);
            # gpsimd queue: everything else, in parallel.
            # x^T loaded column-chunk-major so chunk 0's projections start
            # after ~1/4 of the x DMA, split across the sync and vector DMA
            # queues; weights go on the gpsimd queue in parallel.
            xt_sb = cp.tile([P, KO, S], BF16, tag="xt")
            for qc in range(NQC):
                qs = slice(QCW * qc, QCW * (qc + 1))
                for ko in range(KO):
                    q = nc.sync if ko < KO // 2 else nc.scalar
                    q.dma_start(
                        xt_sb[:, ko, qs], xt[P * ko : P * (ko + 1), qs]
                    )
            wk_sb = cp.tile([P, KO, DH], BF16, tag="wk")
            nc.gpsimd.dma_start(wk_sb[:], wk[:].rearrange("(ko p) m -> p ko m", p=P))
            wv_sb = cp.tile([P, KO, DH], BF16, tag="wv")
            nc.gpsimd.dma_start(wv_sb[:], wv[:].rearrange("(ko p) m -> p ko m", p=P))
            wq_sb = cp.tile([P, KO, DH], BF16, tag="wq")
            nc.gpsimd.dma_start(wq_sb[:], wq[:].rearrange("(ko p) m -> p ko m", p=P))
            cos_sb = cp.tile([P, S], BF16, tag="cos")
            nc.gpsimd.dma_start(cos_sb[:], cosd[:])
            sin_sb = cp.tile([P, S], F32, tag="sin")
            nc.gpsimd.dma_start(sin_sb[:], sind[:])
            sw_sb = cp.tile([P, P], BF16, tag="pswap")
            nc.gpsimd.dma_start(sw_sb[:], pswap[:])
            maskT_sb = cp.tile([P, P], BF16, tag="maskT")
            nc.gpsimd.dma_start(maskT_sb[:], maskTd[:])
            ident_sb = cp.tile([P, P], BF16, tag="ident")
            nc.gpsimd.dma_start(ident_sb[:], identd[:])
            wo_sb = cp.tile([P, 2, D], BF16, tag="wo")
            nc.gpsimd.dma_start(wo_sb[:], wo[:].rearrange("(ko p) n -> p ko n", p=P))

            # V in [s-rows, dims] layout with a ones column per head:
            # [V(64) | 1] -> PV out rows 0..63 = O^T, row 64 = sum(exp)
            v_sb = cp.tile([P, NST, HPC, DK + 1], BF16, tag="v")
            nc.gpsimd.dma_start(
                v_sb[:, :, :, DK], onesd[:].rearrange("p (s h) -> p s h", s=NST)
            )

            kt_sb = cp.tile([P, 2, S], BF16, tag="kt")

            def proj_rope(w_sb, dst, pb, qc):
                """dst <- rope( (x @ W)^T )[128 dims block pb, 512 q cols qc]."""
                qs = slice(QCW * qc, QCW * (qc + 1))
                ps = psP.tile([P, QCW], F32, tag="pp")
                for ko in range(KO):
                    nc.tensor.matmul(
                        ps[:],
                        w_sb[:, ko, P * pb : P * (pb + 1)],
                        xt_sb[:, ko, qs],
                        start=(ko == 0),
                        stop=(ko == KO - 1),
                    )
                raw = rawp.tile([P, QCW], BF16, tag="raw")
                nc.vector.tensor_copy(raw[:], ps[:])
                swp = psP.tile([P, QCW], F32, tag="pp")
                nc.tensor.matmul(swp[:], sw_sb[:], raw[:], start=True, stop=True)
                d1 = ropep.tile([P, QCW], BF16, tag="d1")
                nc.vector.tensor_tensor(d1[:], raw[:], cos_sb[:, qs], MUL)
                d2 = ropep.tile([P, QCW], BF16, tag="d2")
                nc.vector.tensor_tensor(d2[:], swp[:], sin_sb[:, qs], MUL)
                nc.vector.tensor_tensor(dst, d1[:], d2[:], ADD)

            def v_proj(st):
                ps = psP.tile([P, QCW], F32, tag="pp")
                for ko in range(KO):
                    nc.tensor.matmul(
                        ps[:, :DH],
                        xt_sb[:, ko, P * st : P * (st + 1)],
                        wv_sb[:, ko, :],
                        start=(ko == 0),
                        stop=(ko == KO - 1),
                    )
                vv = ps[:, :DH].rearrange("p (h d) -> p h d", h=HPC)
                nc.vector.tensor_copy(v_sb[:, st, :, 0:DK], vv)

            def attn(qc):
                """Attention for q chunk qc; returns the bf16 O^T tile."""
                nst = 4 * qc + 4
                ot_t = otp.tile([P, 2, QCW], BF16, tag="ot")
                for hp in range(2):          # partition block = head pair
                    pvs = []
                    for i in range(2):
                        pvs.append(
                            psV.tile([DK + 1, QCW], F32, tag="pv", name=f"pv{i}")
                        )
                    for g in range(nst // 2):
                        scs = [
                            psS.tile([P, 2, QCW], F32, tag="sc", name=f"sc{i}")
                            for i in range(2)
                        ]
                        rr = []
                        for j in range(2):
                            st = 2 * g + j
                            r = max(P * st - QCW * qc, 0)
                            rr.append(r)
                            for i in range(2):   # head 2hp+i, rows 64i:64i+64
                                nc.tensor.matmul(
                                    scs[i][:, j, r:QCW],
                                    kt_sb[
                                        DK * i : DK * (i + 1),
                                        hp,
                                        P * st : P * (st + 1),
                                    ],
                                    qt_t[DK * i : DK * (i + 1), hp, r:QCW],
                                    start=True,
                                    stop=True,
                                )
                        for j in range(2):
                            st = 2 * g + j
                            r = P * st - QCW * qc
                            if r >= 0:   # diagonal tile: additive triangle mask
                                for i in range(2):
                                    nc.tensor.matmul(
                                        scs[i][:, j, r : r + P],
                                        maskT_sb[:],
                                        ident_sb[:],
                                        start=False,
                                        stop=True,
                                        skip_group_check=True,
                                    )
                        exs = []
                        lo = rr[0]
                        for i in range(2):
                            ex = exp_pool.tile(
                                [P, 2, QCW], BF16, tag="ex", name=f"ex{i}"
                            )
                            exs.append(ex)
                            nc.scalar.activation(
                                ex[:].rearrange("p a b -> p (a b)")[:, lo:],
                                scs[i][:].rearrange("p a b -> p (a b)")[:, lo:],
                                EXP,
                                scale=0.125,
                            )
                        for j in range(2):
                            st = 2 * g + j
                            r = rr[j]
                            for i in range(2):
                                nc.tensor.matmul(
                                    pvs[i][:, r:QCW],
                                    v_sb[:, st, 2 * hp + i, :],
                                    exs[i][:, j, r:QCW],
                                    start=(st == 0),
                                    stop=(st == nst - 1),
                                )
                    # normalize: O / sum(exp).  DVE reciprocal is ~8 cyc/elem,
                    # so spread the 512 sums across partitions before the
                    # reciprocal.  Steady state uses an SBUF-to-SBUF DMA
                    # round-trip (cheap on DVE, latency hidden by the next
                    # chunk's projections); the last chunk has nothing left to
                    # hide under, so it uses the lower-latency DVE 32x32
                    # block-transpose spread instead.
                    for i in range(2):
                        if qc < NQC - 1:
                            den = denp.tile([1, QCW], F32, tag="den")
                            nc.vector.tensor_copy(den[:], pvs[i][DK : DK + 1, :])
                            s128 = denp.tile([P, 4], F32, tag="s128")
                            nc.sync.dma_start(s128[:], den[:])
                            r128 = denp.tile([P, 4], F32, tag="r128")
                            nc.vector.reciprocal(r128[:], s128[:])
                            rc = denp.tile([1, QCW], F32, tag="rc")
                            nc.sync.dma_start(rc[:], r128[:])
                        else:
                            den = denp.tile([32, QCW], F32, tag="dent")
                            nc.vector.tensor_copy(
                                den[0:1, :], pvs[i][DK : DK + 1, :]
                            )
                            dT = denp.tile([32, QCW], F32, tag="dT")
                            nc.vector.transpose(dT[:], den[:])
                            rT = denp.tile([32, QCW], F32, tag="rT")
                            nc.vector.reciprocal(
                                rT[:].rearrange("p (a b) -> p a b", b=32)[:, :, 0],
                                dT[:].rearrange("p (a b) -> p a b", b=32)[:, :, 0],
                            )
                            rc = denp.tile([32, QCW], F32, tag="rct")
                            nc.vector.transpose(rc[:], rT[:])
                            rc = rc[0:1, :]
                        rb = rbp.tile([DK, QCW], F32, tag="rb")
                        nc.gpsimd.partition_broadcast(rb[:], rc[:])
                        nc.vector.tensor_tensor(
                            ot_t[DK * i : DK * (i + 1), hp, :],
                            pvs[i][0:DK, :],
                            rb[:],
                            MUL,
                        )
                return ot_t

            def out_proj(qc, ot_t):
                qs = slice(QCW * qc, QCW * (qc + 1))
                for mt in range(KO):
                    yp = psP.tile([P, QCW], F32, tag="pp")
                    for kb in range(2):
                        nc.tensor.matmul(
                            yp[:],
                            wo_sb[:, kb, P * mt : P * (mt + 1)],
                            ot_t[:, kb, :],
                            start=(kb == 0),
                            stop=(kb == 1),
                        )
                    ys = ysbp.tile([P, QCW], BF16, tag="ys")
                    nc.any.tensor_copy(out=ys[:], in_=yp[:])
                    nc.sync.dma_start(yt[P * mt : P * (mt + 1), qs], ys[:])

            # ---- per-q-chunk schedule ----
            ots = {}
            for qc in range(NQC):
                qs = slice(QCW * qc, QCW * (qc + 1))
                for pb in range(2):
                    proj_rope(wk_sb, kt_sb[:, pb, qs], pb, qc)
                for st in range(4 * qc, 4 * qc + 4):
                    v_proj(st)
                qt_t = qtp.tile([P, 2, QCW], BF16, tag="qt")
                for pb in range(2):
                    proj_rope(wq_sb, qt_t[:, pb, :], pb, qc)
                if qc > 0:
                    out_proj(qc - 1, ots.pop(qc - 1))
                ots[qc] = attn(qc)
            out_proj(NQC - 1, ots.pop(NQC - 1))

    nc.finalize()
    return nc


_NC_CACHE = []
_LAST_IN_MAPS = []


def _rope_tables(token_positions):
    pos = np.asarray(token_positions).astype(np.float32)
    exponent = np.arange(0, DK, 2, dtype=np.float32)
    inv_freq = (1.0 / (10000.0 ** (exponent / DK))).astype(np.float32)
    freqs = pos[:, None] * inv_freq[None, :]          # [S, 32]
    cos64 = np.repeat(np.cos(freqs).T.astype(np.float32), 2, axis=0)  # [64, S]
    sin64 = np.repeat(np.sin(freqs).T.astype(np.float32), 2, axis=0)
    sgn = np.where(np.arange(DK) % 2 == 0, -1.0, 1.0).astype(np.float32)
    sin64 = sin64 * sgn[:, None]
    cos128 = np.tile(cos64, (2, 1)).astype(np.float32)
    sin128 = np.tile(sin64, (2, 1)).astype(np.float32)
    return np.ascontiguousarray(cos128), np.ascontiguousarray(sin128)


def prep_in_maps(x, Wq, Wk, Wv, Wo, token_positions):
    x = np.asarray(x, dtype=np.float32)
    b = x.shape[0]

    cos128, sin128 = _rope_tables(token_positions)
    cos128 = cos128.astype(BF)
    sin128 = sin128.astype(BF)

    psw = np.zeros((P, P), dtype=BF)
    idx = np.arange(P)
    psw[idx, idx ^ 1] = 1.0  # swap adjacent pairs

    # maskT[c, p] = -1e30 where p > c (adds upper-kv triangle mask via
    # psum += maskT.T @ I)
    maskT = np.where(
        np.arange(P)[None, :] > np.arange(P)[:, None], -1e30, 0.0
    ).astype(BF)
    ident = np.eye(P, dtype=BF)
    onesd = np.ones((P, NST * HPC), dtype=BF)

    xts = [np.ascontiguousarray(x[bi].T).astype(BF) for bi in range(b)]
    Wq = np.asarray(Wq, dtype=np.float32).astype(BF)
    Wk = np.asarray(Wk, dtype=np.float32).astype(BF)
    Wv = np.asarray(Wv, dtype=np.float32).astype(BF)
    Wo = np.asarray(Wo, dtype=np.float32).astype(BF)

    in_maps = []
    cpb = NCORES // b  # cores per batch
    for c in range(NCORES):
        bi, g = c // cpb, c % 4
        cs = slice(DH * g, DH * (g + 1))
        in_maps.append(
            {
                "xt": xts[bi],
                "wq": np.ascontiguousarray(Wq[:, cs]),
                "wk": np.ascontiguousarray(Wk[:, cs]),
                "wv": np.ascontiguousarray(Wv[:, cs]),
                "wo": np.ascontiguousarray(Wo[cs, :]),
                "cosd": cos128,
                "sind": sin128,
                "pswap": psw,
                "maskTd": maskT,
                "identd": ident,
                "onesd": onesd,
            }
        )
    return in_maps


def kernel(x, Wq, Wk, Wv, Wo, token_positions):
    b = np.asarray(x).shape[0]
    in_maps = prep_in_maps(x, Wq, Wk, Wv, Wo, token_positions)

    if not _NC_CACHE:
        _NC_CACHE.append(build_nc())
    nc = _NC_CACHE[0]
    _LAST_IN_MAPS.clear()
    _LAST_IN_MAPS.append(in_maps)

    res = run_bass_kernel_spmd(nc, in_maps, list(range(NCORES)), trace=False)

    y = np.zeros((b, S, D), dtype=np.float32)
    cpb = NCORES // b
    for c in range(NCORES):
        y[c // cpb] += res.results[c]["yt"].T.astype(np.float32)
    return y
